# revision 27
# baseline (speedup 1.0000x reference)
"""Trainium2 Bass kernel: 16-head MHA (B=2, S=2048, D=1024) on 8 NeuronCores.

Sharding: core c handles batch c//4 and heads 4*(c%4) .. 4*(c%4)+3
(data parallel over batch, tensor parallel over heads). Q/K/V projections
are column-sharded by head, the output projection is row-sharded; each
core emits a partial (S, D) output and the host sums the 4 partials per
batch.

Schedule (v3): the attention stream processes BOTH heads of a head-group
per step via row-tiled concurrent score matmuls (K=64 each, PE row
groups 0-1 / 2-3), halving score PE time vs the v2 one-head-per-chunk
stream. A chunk is (mt, qc) = (head group, 512-wide q block): 8 chunks
x 16 kb steps. Per step: 2 packed score matmuls into a [128,1024] bank
pair, one 1024-wide exp on ACT (ACT carries ONLY exps - the step clock,
~1147ns), and the lagged PV pair (M=65 with the ones-row denominator
trick). Projections / V' staging / output-projection lumps weave into
the PE slack; all PSUM->SBUF staging copies run on DVE. PSUM: banks 0-3
double-buffer the exp inputs, banks 4-7 hold PV accumulators by chunk
parity; weave matmuls borrow the idle parity pair.
"""

import sys

import numpy as np
import ml_dtypes

if "/opt/trn_rl_repo" not in sys.path:
    sys.path.insert(0, "/opt/trn_rl_repo")

B, S, D = 2, 2048, 1024
H, DK = 16, 64
NCORES = 8
HL = 4            # heads per core
DL = HL * DK      # 256 local projection dims
SCALE = 1.0 / 8.0  # 1/sqrt(DK)
LAG = 6           # PV trails scores by LAG steps
NPT = LAG + 2     # pt (exp output) ring size

_CACHE = {}


def _build_nc():
    import concourse.bass as bass  # noqa: F401
    import concourse.mybir as mybir
    from concourse import bacc, tile

    f32 = mybir.dt.float32
    bf16 = mybir.dt.bfloat16
    AF = mybir.ActivationFunctionType

    nc = bacc.Bacc(None, target_bir_lowering=False, debug=False)
    xqT = nc.declare_dram_parameter("xqT", [D, S], bf16, isOutput=False)
    xkT = nc.declare_dram_parameter("xkT", [D, S], bf16, isOutput=False)
    xvT = nc.declare_dram_parameter("xvT", [D, S], bf16, isOutput=False)
    wqT = nc.declare_dram_parameter("wqT", [D, DL], bf16, isOutput=False)
    wkT = nc.declare_dram_parameter("wkT", [D, DL], bf16, isOutput=False)
    wvT = nc.declare_dram_parameter("wvT", [D, DL], bf16, isOutput=False)
    woT = nc.declare_dram_parameter("woT", [DL, D], bf16, isOutput=False)
    y = nc.declare_dram_parameter("y", [S, D], f32, isOutput=True)

    with tile.TileContext(nc) as tc, \
         tc.tile_pool(name="singles", bufs=1) as singles, \
         tc.tile_pool(name="psum", bufs=1, space="PSUM") as pp, \
         tc.tile_pool(name="dram", bufs=1, space="DRAM") as adr:
        # ---------------- SBUF ----------------
        wq_sb = singles.tile([128, 8, DL], bf16)
        wk_sb = singles.tile([128, 8, DL], bf16)
        wv_sb = singles.tile([128, 8, DL], bf16)
        wo_sb = singles.tile([128, 2, D], bf16)
        qTm = [singles.tile([128, S], bf16, name=f"qT{m}") for m in range(2)]
        kTm = [singles.tile([128, S], bf16, name=f"kT{m}") for m in range(2)]
        atm = [singles.tile([128, S], bf16, name=f"at{m}") for m in range(2)]
        # V' per k-block: [128, mt, 2 heads x (64 v cols + ones)]
        vpst = [singles.tile([128, 2, 130], bf16, name=f"vp{st}")
                for st in range(16)]
        xq_all = singles.tile([128, 8, S], bf16, name="xq")
        xk_all = singles.tile([128, 8, S], bf16, name="xk")
        xv_all = singles.tile([128, 8, S], bf16, name="xv")
        # exp outputs (P^T), ring of NPT
        ptt = [singles.tile([128, 1024], bf16, name=f"pt{i}")
               for i in range(NPT)]
        # norm staging (rotation of 2 chunk-sized sets)
        osb = [singles.tile([65, 1024], f32, name=f"osb{i}") for i in range(2)]
        ddd = [adr.tile([1, 1024], f32, name=f"ddd{i}") for i in range(2)]
        d128 = [singles.tile([128, 8], f32, name=f"d128_{i}") for i in range(2)]
        r128 = [singles.tile([128, 8], f32, name=f"r128_{i}") for i in range(2)]
        rdd = [adr.tile([1, 1024], f32, name=f"rdd{i}") for i in range(2)]
        rec = [singles.tile([64, 1024], f32, name=f"rec{i}") for i in range(2)]
        stg = [singles.tile([64, 512], bf16, name=f"stg{i}") for i in range(2)]
        rrow = singles.tile([1, 1024], f32, name="rrow")
        yo = [singles.tile([128, 1024], f32, name=f"yo{i}") for i in range(3)]

        # ---------------- PSUM: one 8-bank tile, hand-allocated ----------
        P = pp.tile([128, 4096], f32, name="P")

        for st in range(16):
            nc.vector.memset(
                vpst[st].rearrange("p m (h e) -> p m h e", e=65)[:, :, :, 64:65],
                1.0)

        # ---------------- DMA issue order ----------------
        # One dma_start = one descriptor chain = ONE dma engine (~67GB/s),
        # so parallelism needs many small dma_starts spread over the four
        # issuing queues (sync/scalar/vector/gpsimd). First wave (per-ct
        # pieces, 4 queues in parallel): wk+xk-n0 / wq+xq-n0 for the warmup,
        # wv+xv-n0 for the early V' lumps. Later blocks are latency-slack
        # and use fewer, bigger descriptors.
        xkT_r = xkT.rearrange("(c p) s -> p c s", p=128)
        xqT_r = xqT.rearrange("(c p) s -> p c s", p=128)
        xvT_r = xvT.rearrange("(c p) s -> p c s", p=128)

        # Consolidated multi-ct pieces ([128, 8, 512] per n-block), all on
        # sync except the V path on gpsimd - the best-measured balance.
        # Order: warmup set first (wk, xk-n0, wq, xq-n0), then the rest of
        # xk (steps 4-15 consume kb blocks n1-n3), then the later xq blocks.
        def nblk(t, n):
            return t[:, :, n * 512:(n + 1) * 512]

        nc.sync.dma_start(wk_sb[:], wkT.rearrange("(c p) d -> p c d", p=128))
        nc.sync.dma_start(nblk(xk_all, 0), nblk(xkT_r, 0))
        nc.sync.dma_start(wq_sb[:], wqT.rearrange("(c p) d -> p c d", p=128))
        nc.sync.dma_start(nblk(xq_all, 0), nblk(xqT_r, 0))
        nc.gpsimd.dma_start(out=wv_sb[:],
                            in_=wvT.rearrange("(c p) d -> p c d", p=128))
        for n in range(4):
            nc.gpsimd.dma_start(out=nblk(xv_all, n), in_=nblk(xvT_r, n))
        for n in range(1, 4):
            nc.sync.dma_start(nblk(xk_all, n), nblk(xkT_r, n))
        # xq-n1 rides the gpsimd queue (idle after xv, ~39us) so the q0-qc1
        # projection at step 13 never waits behind the sync queue's xk blocks
        nc.gpsimd.dma_start(out=nblk(xq_all, 1), in_=nblk(xqT_r, 1))
        for n in range(2, 4):
            nc.sync.dma_start(nblk(xq_all, n), nblk(xqT_r, n))
        nc.sync.dma_start(wo_sb[:], woT.rearrange("(g p) d -> p g d", p=128))

        # ---------------- weave closures ----------------
        def vlump(st, half, col):
            # one mt-half of V'[st]: 8 ct matmuls (N=128) + DVE stage copy
            def go():
                vt = P[:, col:col + 128]
                for ct in range(8):
                    nc.tensor.matmul(
                        vt,
                        lhsT=xv_all[:, ct, st * 128:(st + 1) * 128],
                        rhs=wv_sb[:, ct, half * 128:(half + 1) * 128],
                        start=(ct == 0), stop=(ct == 7),
                    )
                dst = vpst[st].rearrange(
                    "p m (h e) -> p m h e", e=65)[:, half, :, 0:64]
                nc.vector.tensor_copy(dst, vt.rearrange("p (h d) -> p h d", d=64))
            return go

        def projlump(w_sb, x_all, dst, mt, n, col):
            def go():
                acc = P[:, col:col + 512]
                for ct in range(8):
                    nc.tensor.matmul(
                        acc,
                        lhsT=w_sb[:, ct, mt * 128:(mt + 1) * 128],
                        rhs=x_all[:, ct, n * 512:(n + 1) * 512],
                        start=(ct == 0), stop=(ct == 7),
                    )
                nc.vector.tensor_copy(dst[:, n * 512:(n + 1) * 512], acc)
            return go

        yo_i = [0]

        def ylump(st, col, tail=False):
            # output rows st*128..+128: 4 matmuls (atm0 K=128 start,
            # atm1 K=128 stop) per 512-col half, stage copy, DMA out.
            # Tail lumps parallelize: copies alternate DVE/ACT (ACT is free
            # once exps end) and the y DMAs split across sync/scalar queues.
            def go():
                for ec in range(2):
                    out = P[:, col + ec * 512:col + ec * 512 + 512]
                    nc.tensor.matmul(
                        out,
                        lhsT=atm[0][:, st * 128:(st + 1) * 128],
                        rhs=wo_sb[:, 0, ec * 512:(ec + 1) * 512],
                        start=True, stop=False,
                    )
                for ec in range(2):
                    out = P[:, col + ec * 512:col + ec * 512 + 512]
                    nc.tensor.matmul(
                        out,
                        lhsT=atm[1][:, st * 128:(st + 1) * 128],
                        rhs=wo_sb[:, 1, ec * 512:(ec + 1) * 512],
                        start=False, stop=True,
                    )
                r = yo_i[0] % 3
                yo_i[0] += 1
                if tail and st % 2 == 1:
                    nc.scalar.activation(yo[r][:], P[:, col:col + 1024],
                                         AF.Copy)
                else:
                    nc.vector.tensor_copy(yo[r][:], P[:, col:col + 1024])
                if tail:
                    nc.sync.dma_start(y[st * 128:(st + 1) * 128, 0:512],
                                      yo[r][:, 0:512])
                    nc.scalar.dma_start(y[st * 128:(st + 1) * 128, 512:1024],
                                        yo[r][:, 512:1024])
                else:
                    nc.sync.dma_start(y[st * 128:(st + 1) * 128, :], yo[r][:])
            return go

        # ---------------- warmup: k0-n0 + q0-qc0 ----------------
        # banks 6,7 (parity-1 PV pair is free until step 22). k-MMs first:
        # wk/xk-n0 arrive ahead of wq/xq-n0 on their queues.
        for ct in range(8):
            nc.tensor.matmul(
                P[:, 3072:3584],
                lhsT=wk_sb[:, ct, 0:128],
                rhs=xk_all[:, ct, 0:512],
                start=(ct == 0), stop=(ct == 7),
            )
        nc.vector.tensor_copy(kTm[0][:, 0:512], P[:, 3072:3584])
        for ct in range(8):
            nc.tensor.matmul(
                P[:, 3584:4096],
                lhsT=wq_sb[:, ct, 0:128],
                rhs=xq_all[:, ct, 0:512],
                start=(ct == 0), stop=(ct == 7),
            )
        nc.vector.tensor_copy(qTm[0][:, 0:512], P[:, 3584:4096])

        # ---------------- weave schedule: step -> [closures] -------------
        # chunk c = chunks[c] = (mt, qc); parity banks: base 2048+1024*(c%2)
        chunks = [(0, 0), (0, 1), (1, 0), (1, 1),
                  (0, 2), (1, 2), (0, 3), (1, 3)]
        nsteps = 16 * len(chunks)

        W = {}

        def add(step, fn):
            W.setdefault(step, []).append(fn)

        def free_base(p, w=1):
            # base column of a free 2-bank PSUM pair at step p. Chunk c's
            # own parity banks are busy from step 16c+6 (PV kb0) until
            # ~16(c+1)+7 (osb copy). Safe: own parity at p%16 <= 3,
            # the other parity at p%16 >= 9.
            c = p // 16
            if p % 16 <= 3:
                return 2048 + 1024 * (c % 2)
            assert p % 16 >= 9, f"no free psum at step {p}"
            return 2048 + 1024 * ((c + 1) % 2)

        # k0 n1..n3 (kb 4..15 of chunks 0,1) - deadlines steps 4, 8, 12
        add(0, projlump(wk_sb, xk_all, kTm[0], 0, 1, free_base(0)))
        add(2, projlump(wk_sb, xk_all, kTm[0], 0, 2, free_base(2)))
        add(9, projlump(wk_sb, xk_all, kTm[0], 0, 3, free_base(9)))
        # V' mt0 st3..15, just-in-time (needed at step st+6); steps 4..8
        # fall in the psum guard band, so those lumps double up on 9..13
        mt0_sched = [(0, 0), (1, 1), (2, 2), (3, 3), (4, 3), (5, 9),
                     (6, 9), (7, 9), (8, 10), (9, 10), (10, 11), (11, 11),
                     (12, 12), (13, 12), (14, 13), (15, 13)]
        for st, q in mt0_sched:
            add(q, vlump(st, 0, free_base(q) + 512 + 128 * (st % 4)))
        # q0 qc1 - deadline step 16
        add(13, projlump(wq_sb, xq_all, qTm[0], 0, 1, free_base(13)))
        # k1 n0..n3 - deadlines 32..44; q1 qc0 by 32, qc1 by 48
        add(16, projlump(wk_sb, xk_all, kTm[1], 1, 0, free_base(16)))
        add(18, projlump(wk_sb, xk_all, kTm[1], 1, 1, free_base(18)))
        add(25, projlump(wk_sb, xk_all, kTm[1], 1, 2, free_base(25)))
        add(27, projlump(wk_sb, xk_all, kTm[1], 1, 3, free_base(27)))
        add(29, projlump(wq_sb, xq_all, qTm[1], 1, 0, free_base(29)))
        add(41, projlump(wq_sb, xq_all, qTm[1], 1, 1, free_base(41)))
        # V' mt1 st0..15 - deadlines 38+st (chunk 2). Window: steps 32-51
        for st, q in [(0, 32), (1, 33), (2, 34), (3, 35), (4, 41), (5, 42),
                      (6, 43), (7, 44), (8, 45), (9, 46), (10, 47), (11, 48),
                      (12, 49), (13, 50), (14, 51), (15, 51)]:
            add(q, vlump(st, 1, free_base(q) + 512 + 128 * (st % 4)))
        # remaining q projections
        add(57, projlump(wq_sb, xq_all, qTm[0], 0, 2, free_base(57)))   # dl 64
        add(73, projlump(wq_sb, xq_all, qTm[1], 1, 2, free_base(73)))   # dl 80
        add(89, projlump(wq_sb, xq_all, qTm[0], 0, 3, free_base(89)))   # dl 96
        add(105, projlump(wq_sb, xq_all, qTm[1], 1, 3, free_base(105)))  # dl 112
        # output rows: qc0 ready ~step 61, qc1 ~77, qc2 ~109, qc3 tail
        for j, q in enumerate([62, 63, 73, 75]):
            add(q, ylump(j, free_base(q)))
        for j, q in enumerate([78, 80, 82, 91]):
            add(q, ylump(4 + j, free_base(q)))
        for j, q in enumerate([109, 110, 121, 123]):
            add(q, ylump(8 + j, free_base(q)))

        # ---------------- norm chain ----------------
        def ot_col(c):
            return 2048 + 1024 * (c % 2)

        def norm1(c):
            # copy O^T (2 heads x [65,512], contiguous bank pair) + dens
            # to SBUF, then 1/den via DRAM-reshape so the reciprocal runs
            # 128 lanes wide, and a broadcast read back - off the PE stream
            r = c % 2
            nc.vector.tensor_copy(osb[r][:], P[0:65, ot_col(c):ot_col(c) + 1024])
            nc.gpsimd.dma_start(out=ddd[r][:], in_=osb[r][64:65, :])
            nc.gpsimd.dma_start(
                out=d128[r][:],
                in_=ddd[r].rearrange("a (p j) -> (a p) j", j=8))
            nc.vector.reciprocal(r128[r][:], d128[r][:])
            nc.gpsimd.dma_start(
                out=rdd[r].rearrange("a (p j) -> (a p) j", j=8),
                in_=r128[r][:])
            nc.gpsimd.dma_start(out=rec[r][:],
                                in_=rdd[r].broadcast_to([64, 1024]))

        def norm2(c):
            # 6 steps later: normalize both heads (SBUF only) and ship to
            # atm: head hh -> atm[mt] partitions 64*hh..+64, column block qc
            mt, qc = chunks[c]
            r = c % 2
            for hh in range(2):
                s = stg[hh]
                nc.vector.tensor_mul(
                    s[:],
                    osb[r][0:64, hh * 512:(hh + 1) * 512],
                    rec[r][:, hh * 512:(hh + 1) * 512])
                nc.gpsimd.dma_start(
                    out=atm[mt][64 * hh:64 * hh + 64, qc * 512:(qc + 1) * 512],
                    in_=s[:])

        norm2_at = {}
        pv_at = {}
        for c in range(len(chunks)):
            # the last chunk's PV runs at lag 2 so its norm chain (the only
            # thing gating the final 4 output rows) starts ~4 steps earlier
            lag = 2 if c == len(chunks) - 1 else LAG
            for kb in range(16):
                pv_at.setdefault(16 * c + kb + lag, []).append((c, kb))

        def emit_step(p):
            for c, kb in pv_at.pop(p, ()):
                mt, qc = chunks[c]
                pt = ptt[(16 * c + kb) % NPT]
                base = ot_col(c)
                for hh in range(2):
                    nc.tensor.matmul(
                        P[0:65, base + hh * 512:base + hh * 512 + 512],
                        lhsT=vpst[kb][:, mt, hh * 65:(hh + 1) * 65],
                        rhs=pt[:, hh * 512:(hh + 1) * 512],
                        start=(kb == 0), stop=(kb == 15),
                    )
                if kb == 15:
                    norm1(c)
                    norm2_at.setdefault(p + 6, []).append(c)
            for ent in norm2_at.pop(p, ()):
                norm2(ent)
            # scores (both heads, row-tiled concurrent) + exp
            if p < nsteps:
                c, kb = p // 16, p % 16
                mt, qc = chunks[c]
                base = (p % 2) * 1024
                for hh in range(2):
                    po = 64 * hh
                    nc.tensor.matmul(
                        P[:, base + hh * 512: base + hh * 512 + 512],
                        lhsT=kTm[mt][po:po + 64, kb * 128:(kb + 1) * 128],
                        rhs=qTm[mt][po:po + 64, qc * 512:(qc + 1) * 512],
                        start=True, stop=True,
                    )
                nc.scalar.activation(ptt[p % NPT][:], P[:, base:base + 1024],
                                     AF.Exp, scale=SCALE)
            # weave
            for fn in W.pop(p, ()):
                fn()

        for p in range(nsteps + LAG):
            emit_step(p)

        # ---------------- tail: rows 12..15 ----------
        norm2(7)
        for j, st in enumerate((12, 13, 14, 15)):
            ylump(st, (0, 1024, 2048, 3072)[j], tail=True)()
        norm2_at.clear()

    nc.finalize()
    return nc


def get_nc():
    if "nc" not in _CACHE:
        _CACHE["nc"] = _build_nc()
    return _CACHE["nc"]


def make_in_maps(query, key, value, W_q, W_k, W_v, W_o):
    bf = ml_dtypes.bfloat16

    def t(a):  # contiguous transpose + bf16 cast
        return np.ascontiguousarray(np.asarray(a, np.float32).T).astype(bf)

    xq = {b: t(query[b]) for b in range(B)}
    xk = {b: t(key[b]) for b in range(B)}
    xv = {b: t(value[b]) for b in range(B)}
    W_q, W_k, W_v, W_o = (np.asarray(w, np.float32) for w in (W_q, W_k, W_v, W_o))
    wq = {g: t(W_q[g * DL:(g + 1) * DL, :]) for g in range(4)}
    wk = {g: t(W_k[g * DL:(g + 1) * DL, :]) for g in range(4)}
    wv = {g: t(W_v[g * DL:(g + 1) * DL, :]) for g in range(4)}
    wo = {g: t(W_o[:, g * DL:(g + 1) * DL]) for g in range(4)}

    in_maps = []
    for c in range(NCORES):
        b, g = divmod(c, 4)
        in_maps.append({
            "xqT": xq[b], "xkT": xk[b], "xvT": xv[b],
            "wqT": wq[g], "wkT": wk[g], "wvT": wv[g], "woT": wo[g],
        })
    return in_maps


def combine_outputs(results):
    """results: list of per-core dicts with 'y' -> full (B, S, D) output."""
    outs = [np.asarray(r["y"], np.float32) for r in results]
    return np.stack([
        outs[0] + outs[1] + outs[2] + outs[3],
        outs[4] + outs[5] + outs[6] + outs[7],
    ]).astype(np.float32)


def _exec_cached(nc, in_maps):
    """run_bass_via_pjrt with the jitted executable cached across calls."""
    import jax
    import jax.numpy as jnp  # noqa: F401
    from jax.sharding import Mesh, PartitionSpec
    from jax.experimental.shard_map import shard_map
    import concourse.mybir as mybir
    from concourse import bass2jax

    if "exec" not in _CACHE:
        bass2jax.install_neuronx_cc_hook()
        partition_name = (nc.partition_id_tensor.name
                          if nc.partition_id_tensor else None)
        in_names, out_names, out_avals = [], [], []
        for alloc in nc.m.functions[0].allocations:
            if not isinstance(alloc, mybir.MemoryLocationSet):
                continue
            name = alloc.memorylocations[0].name
            if alloc.kind == "ExternalInput":
                if name != partition_name:
                    in_names.append(name)
            elif alloc.kind == "ExternalOutput":
                out_avals.append(jax.core.ShapedArray(
                    tuple(alloc.tensor_shape), mybir.dt.np(alloc.dtype)))
                out_names.append(name)
        n_params = len(in_names)
        all_names = in_names + out_names
        if partition_name is not None:
            all_names.append(partition_name)
        donate = tuple(range(n_params, n_params + len(out_names)))

        def _body(*args):
            operands = list(args)
            if partition_name is not None:
                operands.append(bass2jax.partition_id_tensor())
            outs = bass2jax._bass_exec_p.bind(
                *operands,
                out_avals=tuple(out_avals),
                in_names=tuple(all_names),
                out_names=tuple(out_names),
                lowering_input_output_aliases=(),
                sim_require_finite=True,
                sim_require_nnan=True,
                nc=nc,
            )
            return tuple(outs)

        mesh = Mesh(np.asarray(jax.devices()[:NCORES]), ("core",))
        specs = (PartitionSpec("core"),) * (n_params + len(out_names))
        out_specs = (PartitionSpec("core"),) * len(out_names)
        _CACHE["exec"] = (
            jax.jit(shard_map(_body, mesh=mesh, in_specs=specs,
                              out_specs=out_specs, check_rep=False),
                    donate_argnums=donate, keep_unused=True),
            in_names, out_names, out_avals,
        )

    sharded, in_names, out_names, out_avals = _CACHE["exec"]
    concat_in = [
        np.concatenate([np.asarray(in_maps[c][name]) for c in range(NCORES)],
                       axis=0)
        for name in in_names
    ]
    concat_zeros = [
        np.zeros((NCORES * a.shape[0], *a.shape[1:]), a.dtype)
        for a in out_avals
    ]
    out_arrs = sharded(*concat_in, *concat_zeros)
    return [
        {name: np.asarray(out_arrs[i]).reshape(
            NCORES, *out_avals[i].shape)[c]
         for i, name in enumerate(out_names)}
        for c in range(NCORES)
    ]


def kernel(query, key, value, W_q, W_k, W_v, W_o):
    nc = get_nc()
    in_maps = make_in_maps(query, key, value, W_q, W_k, W_v, W_o)
    try:
        results = _exec_cached(nc, in_maps)
    except Exception:
        from concourse.bass_utils import run_bass_kernel_spmd
        _CACHE.pop("exec", None)
        results = run_bass_kernel_spmd(nc, in_maps, list(range(NCORES))).results
    return combine_outputs(results)


# revision 29
# speedup vs baseline: 1.1709x; 1.1709x over previous
"""Trainium2 Bass kernel: 16-head MHA (B=2, S=2048, D=1024) on 8 NeuronCores.

Sharding: core c handles batch c//4 and heads 4*(c%4) .. 4*(c%4)+3
(data parallel over batch, tensor parallel over heads). Q/K/V projections
are column-sharded by head, the output projection is row-sharded; each
core emits a partial (S, D) output and the host sums the 4 partials per
batch.

Schedule (v3): the attention stream processes BOTH heads of a head-group
per step via row-tiled concurrent score matmuls (K=64 each, PE row
groups 0-1 / 2-3), halving score PE time vs the v2 one-head-per-chunk
stream. A chunk is (mt, qc) = (head group, 512-wide q block): 8 chunks
x 16 kb steps. Per step: 2 packed score matmuls into a [128,1024] bank
pair, one 1024-wide exp on ACT (ACT carries ONLY exps - the step clock,
~1147ns), and the lagged PV pair (M=65 with the ones-row denominator
trick). Projections / V' staging / output-projection lumps weave into
the PE slack; all PSUM->SBUF staging copies run on DVE. PSUM: banks 0-3
double-buffer the exp inputs, banks 4-7 hold PV accumulators by chunk
parity; weave matmuls borrow the idle parity pair.
"""

import sys

import numpy as np
import ml_dtypes

if "/opt/trn_rl_repo" not in sys.path:
    sys.path.insert(0, "/opt/trn_rl_repo")

B, S, D = 2, 2048, 1024
H, DK = 16, 64
NCORES = 8
HL = 4            # heads per core
DL = HL * DK      # 256 local projection dims
SCALE = 1.0 / 8.0  # 1/sqrt(DK)
LAG = 6           # PV trails scores by LAG steps
NPT = LAG + 2     # pt (exp output) ring size

_CACHE = {}


def _build_nc():
    import concourse.bass as bass  # noqa: F401
    import concourse.mybir as mybir
    from concourse import bacc, tile

    f32 = mybir.dt.float32
    bf16 = mybir.dt.bfloat16
    AF = mybir.ActivationFunctionType

    nc = bacc.Bacc(None, target_bir_lowering=False, debug=False)
    xqT = nc.declare_dram_parameter("xqT", [D, S], bf16, isOutput=False)
    xkT = nc.declare_dram_parameter("xkT", [D, S], bf16, isOutput=False)
    xvT = nc.declare_dram_parameter("xvT", [D, S], bf16, isOutput=False)
    wqT = nc.declare_dram_parameter("wqT", [D, DL], bf16, isOutput=False)
    wkT = nc.declare_dram_parameter("wkT", [D, DL], bf16, isOutput=False)
    wvT = nc.declare_dram_parameter("wvT", [D, DL], bf16, isOutput=False)
    woT = nc.declare_dram_parameter("woT", [DL, D], bf16, isOutput=False)
    y = nc.declare_dram_parameter("y", [S, D], f32, isOutput=True)

    with tile.TileContext(nc) as tc, \
         tc.tile_pool(name="singles", bufs=1) as singles, \
         tc.tile_pool(name="psum", bufs=1, space="PSUM") as pp, \
         tc.tile_pool(name="dram", bufs=1, space="DRAM") as adr:
        # ---------------- SBUF ----------------
        wq_sb = singles.tile([128, 8, DL], bf16)
        wk_sb = singles.tile([128, 8, DL], bf16)
        wv_sb = singles.tile([128, 8, DL], bf16)
        wo_sb = singles.tile([128, 2, D], bf16)
        qTm = [singles.tile([128, S], bf16, name=f"qT{m}") for m in range(2)]
        kTm = [singles.tile([128, S], bf16, name=f"kT{m}") for m in range(2)]
        atm = [singles.tile([128, S], bf16, name=f"at{m}") for m in range(2)]
        # V' per k-block: [128, mt, 2 heads x (64 v cols + ones)]
        vpst = [singles.tile([128, 2, 130], bf16, name=f"vp{st}")
                for st in range(16)]
        xq_all = singles.tile([128, 8, S], bf16, name="xq")
        xk_all = singles.tile([128, 8, S], bf16, name="xk")
        xv_all = singles.tile([128, 8, S], bf16, name="xv")
        # exp outputs (P^T), ring of NPT
        ptt = [singles.tile([128, 1024], bf16, name=f"pt{i}")
               for i in range(NPT)]
        # norm staging (rotation of 2 chunk-sized sets)
        osb = [singles.tile([65, 1024], f32, name=f"osb{i}") for i in range(2)]
        ddd = [adr.tile([1, 1024], f32, name=f"ddd{i}") for i in range(2)]
        d128 = [singles.tile([128, 8], f32, name=f"d128_{i}") for i in range(2)]
        r128 = [singles.tile([128, 8], f32, name=f"r128_{i}") for i in range(2)]
        rdd = [adr.tile([1, 1024], f32, name=f"rdd{i}") for i in range(2)]
        rec = [singles.tile([64, 1024], f32, name=f"rec{i}") for i in range(2)]
        stg = [singles.tile([64, 512], bf16, name=f"stg{i}") for i in range(2)]
        rrow = singles.tile([1, 1024], f32, name="rrow")
        yo = [singles.tile([128, 1024], f32, name=f"yo{i}") for i in range(3)]

        # ---------------- PSUM: one 8-bank tile, hand-allocated ----------
        P = pp.tile([128, 4096], f32, name="P")

        for st in range(16):
            nc.vector.memset(
                vpst[st].rearrange("p m (h e) -> p m h e", e=65)[:, :, :, 64:65],
                1.0)

        # ---------------- DMA issue order ----------------
        # One dma_start = one descriptor chain = ONE dma engine (~67GB/s),
        # so parallelism needs many small dma_starts spread over the four
        # issuing queues (sync/scalar/vector/gpsimd). First wave (per-ct
        # pieces, 4 queues in parallel): wk+xk-n0 / wq+xq-n0 for the warmup,
        # wv+xv-n0 for the early V' lumps. Later blocks are latency-slack
        # and use fewer, bigger descriptors.
        xkT_r = xkT.rearrange("(c p) s -> p c s", p=128)
        xqT_r = xqT.rearrange("(c p) s -> p c s", p=128)
        xvT_r = xvT.rearrange("(c p) s -> p c s", p=128)

        # Consolidated multi-ct pieces ([128, 8, 512] per n-block), all on
        # sync except the V path on gpsimd - the best-measured balance.
        # Order: warmup set first (wk, xk-n0, wq, xq-n0), then the rest of
        # xk (steps 4-15 consume kb blocks n1-n3), then the later xq blocks.
        def nblk(t, n):
            return t[:, :, n * 512:(n + 1) * 512]

        nc.sync.dma_start(wk_sb[:], wkT.rearrange("(c p) d -> p c d", p=128))
        nc.sync.dma_start(nblk(xk_all, 0), nblk(xkT_r, 0))
        nc.sync.dma_start(wq_sb[:], wqT.rearrange("(c p) d -> p c d", p=128))
        nc.sync.dma_start(nblk(xq_all, 0), nblk(xqT_r, 0))
        nc.gpsimd.dma_start(out=wv_sb[:],
                            in_=wvT.rearrange("(c p) d -> p c d", p=128))
        for n in range(4):
            nc.gpsimd.dma_start(out=nblk(xv_all, n), in_=nblk(xvT_r, n))
        for n in range(1, 4):
            nc.sync.dma_start(nblk(xk_all, n), nblk(xkT_r, n))
        for n in range(1, 4):
            nc.sync.dma_start(nblk(xq_all, n), nblk(xqT_r, n))
        nc.sync.dma_start(wo_sb[:], woT.rearrange("(g p) d -> p g d", p=128))

        # ---------------- weave closures ----------------
        def vlump(st, half, col):
            # one mt-half of V'[st]: 8 ct matmuls (N=128) + DVE stage copy
            def go():
                vt = P[:, col:col + 128]
                for ct in range(8):
                    nc.tensor.matmul(
                        vt,
                        lhsT=xv_all[:, ct, st * 128:(st + 1) * 128],
                        rhs=wv_sb[:, ct, half * 128:(half + 1) * 128],
                        start=(ct == 0), stop=(ct == 7),
                    )
                dst = vpst[st].rearrange(
                    "p m (h e) -> p m h e", e=65)[:, half, :, 0:64]
                nc.vector.tensor_copy(dst, vt.rearrange("p (h d) -> p h d", d=64))
            return go

        def projlump(w_sb, x_all, dst, mt, n, col):
            def go():
                acc = P[:, col:col + 512]
                for ct in range(8):
                    nc.tensor.matmul(
                        acc,
                        lhsT=w_sb[:, ct, mt * 128:(mt + 1) * 128],
                        rhs=x_all[:, ct, n * 512:(n + 1) * 512],
                        start=(ct == 0), stop=(ct == 7),
                    )
                nc.vector.tensor_copy(dst[:, n * 512:(n + 1) * 512], acc)
            return go

        yo_i = [0]

        def ylump(st, col, tail=False):
            # output rows st*128..+128: 4 matmuls (atm0 K=128 start,
            # atm1 K=128 stop) per 512-col half, stage copy, DMA out.
            # Tail lumps parallelize: copies alternate DVE/ACT (ACT is free
            # once exps end) and the y DMAs split across sync/scalar queues.
            def go():
                for ec in range(2):
                    out = P[:, col + ec * 512:col + ec * 512 + 512]
                    nc.tensor.matmul(
                        out,
                        lhsT=atm[0][:, st * 128:(st + 1) * 128],
                        rhs=wo_sb[:, 0, ec * 512:(ec + 1) * 512],
                        start=True, stop=False,
                    )
                for ec in range(2):
                    out = P[:, col + ec * 512:col + ec * 512 + 512]
                    nc.tensor.matmul(
                        out,
                        lhsT=atm[1][:, st * 128:(st + 1) * 128],
                        rhs=wo_sb[:, 1, ec * 512:(ec + 1) * 512],
                        start=False, stop=True,
                    )
                r = yo_i[0] % 3
                yo_i[0] += 1
                if tail and st % 2 == 1:
                    nc.scalar.activation(yo[r][:], P[:, col:col + 1024],
                                         AF.Copy)
                else:
                    nc.vector.tensor_copy(yo[r][:], P[:, col:col + 1024])
                if tail:
                    nc.sync.dma_start(y[st * 128:(st + 1) * 128, 0:512],
                                      yo[r][:, 0:512])
                    nc.scalar.dma_start(y[st * 128:(st + 1) * 128, 512:1024],
                                        yo[r][:, 512:1024])
                else:
                    nc.sync.dma_start(y[st * 128:(st + 1) * 128, :], yo[r][:])
            return go

        # ---------------- warmup: k0-n0 + q0-qc0 ----------------
        # banks 6,7 (parity-1 PV pair is free until step 22). k-MMs first:
        # wk/xk-n0 arrive ahead of wq/xq-n0 on their queues.
        for ct in range(8):
            nc.tensor.matmul(
                P[:, 3072:3584],
                lhsT=wk_sb[:, ct, 0:128],
                rhs=xk_all[:, ct, 0:512],
                start=(ct == 0), stop=(ct == 7),
            )
        nc.vector.tensor_copy(kTm[0][:, 0:512], P[:, 3072:3584])
        for ct in range(8):
            nc.tensor.matmul(
                P[:, 3584:4096],
                lhsT=wq_sb[:, ct, 0:128],
                rhs=xq_all[:, ct, 0:512],
                start=(ct == 0), stop=(ct == 7),
            )
        nc.vector.tensor_copy(qTm[0][:, 0:512], P[:, 3584:4096])

        # ---------------- weave schedule: step -> [closures] -------------
        # chunk c = chunks[c] = (mt, qc); parity banks: base 2048+1024*(c%2)
        chunks = [(0, 0), (0, 1), (1, 0), (1, 1),
                  (0, 2), (1, 2), (0, 3), (1, 3)]
        nsteps = 16 * len(chunks)

        W = {}

        def add(step, fn):
            W.setdefault(step, []).append(fn)

        def free_base(p, w=1):
            # base column of a free 2-bank PSUM pair at step p. Chunk c's
            # own parity banks are busy from step 16c+6 (PV kb0) until
            # ~16(c+1)+7 (osb copy). Safe: own parity at p%16 <= 3,
            # the other parity at p%16 >= 9.
            c = p // 16
            if p % 16 <= 3:
                return 2048 + 1024 * (c % 2)
            assert p % 16 >= 9, f"no free psum at step {p}"
            return 2048 + 1024 * ((c + 1) % 2)

        # k0 n1..n3 (kb 4..15 of chunks 0,1) - deadlines steps 4, 8, 12
        add(0, projlump(wk_sb, xk_all, kTm[0], 0, 1, free_base(0)))
        add(2, projlump(wk_sb, xk_all, kTm[0], 0, 2, free_base(2)))
        add(9, projlump(wk_sb, xk_all, kTm[0], 0, 3, free_base(9)))
        # V' mt0 st3..15, just-in-time (needed at step st+6); steps 4..8
        # fall in the psum guard band, so those lumps double up on 9..13
        mt0_sched = [(0, 0), (1, 1), (2, 2), (3, 3), (4, 3), (5, 9),
                     (6, 9), (7, 9), (8, 10), (9, 10), (10, 11), (11, 11),
                     (12, 12), (13, 12), (14, 13), (15, 13)]
        for st, q in mt0_sched:
            add(q, vlump(st, 0, free_base(q) + 512 + 128 * (st % 4)))
        # q0 qc1 - deadline step 16
        add(13, projlump(wq_sb, xq_all, qTm[0], 0, 1, free_base(13)))
        # k1 n0..n3 - deadlines 32..44; q1 qc0 by 32, qc1 by 48
        add(16, projlump(wk_sb, xk_all, kTm[1], 1, 0, free_base(16)))
        add(18, projlump(wk_sb, xk_all, kTm[1], 1, 1, free_base(18)))
        add(25, projlump(wk_sb, xk_all, kTm[1], 1, 2, free_base(25)))
        add(27, projlump(wk_sb, xk_all, kTm[1], 1, 3, free_base(27)))
        add(29, projlump(wq_sb, xq_all, qTm[1], 1, 0, free_base(29)))
        add(41, projlump(wq_sb, xq_all, qTm[1], 1, 1, free_base(41)))
        # V' mt1 st0..15 - deadlines 38+st (chunk 2). Window: steps 32-51
        for st, q in [(0, 32), (1, 33), (2, 34), (3, 35), (4, 41), (5, 42),
                      (6, 43), (7, 44), (8, 45), (9, 46), (10, 47), (11, 48),
                      (12, 49), (13, 50), (14, 51), (15, 51)]:
            add(q, vlump(st, 1, free_base(q) + 512 + 128 * (st % 4)))
        # remaining q projections
        add(57, projlump(wq_sb, xq_all, qTm[0], 0, 2, free_base(57)))   # dl 64
        add(73, projlump(wq_sb, xq_all, qTm[1], 1, 2, free_base(73)))   # dl 80
        add(89, projlump(wq_sb, xq_all, qTm[0], 0, 3, free_base(89)))   # dl 96
        add(105, projlump(wq_sb, xq_all, qTm[1], 1, 3, free_base(105)))  # dl 112
        # output rows: qc0 ready ~step 61, qc1 ~77, qc2 ~109, qc3 tail
        for j, q in enumerate([62, 63, 73, 75]):
            add(q, ylump(j, free_base(q)))
        for j, q in enumerate([78, 80, 82, 91]):
            add(q, ylump(4 + j, free_base(q)))
        for j, q in enumerate([109, 110, 121, 123]):
            add(q, ylump(8 + j, free_base(q)))

        # ---------------- norm chain ----------------
        def ot_col(c):
            return 2048 + 1024 * (c % 2)

        def norm1(c):
            # copy O^T (2 heads x [65,512], contiguous bank pair) + dens
            # to SBUF, then 1/den via DRAM-reshape so the reciprocal runs
            # 128 lanes wide, and a broadcast read back - off the PE stream
            r = c % 2
            nc.vector.tensor_copy(osb[r][:], P[0:65, ot_col(c):ot_col(c) + 1024])
            nc.gpsimd.dma_start(out=ddd[r][:], in_=osb[r][64:65, :])
            nc.gpsimd.dma_start(
                out=d128[r][:],
                in_=ddd[r].rearrange("a (p j) -> (a p) j", j=8))
            nc.vector.reciprocal(r128[r][:], d128[r][:])
            nc.gpsimd.dma_start(
                out=rdd[r].rearrange("a (p j) -> (a p) j", j=8),
                in_=r128[r][:])
            nc.gpsimd.dma_start(out=rec[r][:],
                                in_=rdd[r].broadcast_to([64, 1024]))

        def norm2(c):
            # 6 steps later: normalize both heads (SBUF only) and ship to
            # atm: head hh -> atm[mt] partitions 64*hh..+64, column block qc
            mt, qc = chunks[c]
            r = c % 2
            for hh in range(2):
                s = stg[hh]
                nc.vector.tensor_mul(
                    s[:],
                    osb[r][0:64, hh * 512:(hh + 1) * 512],
                    rec[r][:, hh * 512:(hh + 1) * 512])
                nc.gpsimd.dma_start(
                    out=atm[mt][64 * hh:64 * hh + 64, qc * 512:(qc + 1) * 512],
                    in_=s[:])

        norm2_at = {}
        pv_at = {}
        for c in range(len(chunks)):
            # the last chunk's PV runs at lag 2 so its norm chain (the only
            # thing gating the final 4 output rows) starts ~4 steps earlier
            lag = 2 if c == len(chunks) - 1 else LAG
            for kb in range(16):
                pv_at.setdefault(16 * c + kb + lag, []).append((c, kb))

        def emit_step(p):
            for c, kb in pv_at.pop(p, ()):
                mt, qc = chunks[c]
                pt = ptt[(16 * c + kb) % NPT]
                base = ot_col(c)
                for hh in range(2):
                    nc.tensor.matmul(
                        P[0:65, base + hh * 512:base + hh * 512 + 512],
                        lhsT=vpst[kb][:, mt, hh * 65:(hh + 1) * 65],
                        rhs=pt[:, hh * 512:(hh + 1) * 512],
                        start=(kb == 0), stop=(kb == 15),
                    )
                if kb == 15:
                    norm1(c)
                    norm2_at.setdefault(p + 6, []).append(c)
            for ent in norm2_at.pop(p, ()):
                norm2(ent)
            # scores (both heads, row-tiled concurrent) + exp
            if p < nsteps:
                c, kb = p // 16, p % 16
                mt, qc = chunks[c]
                base = (p % 2) * 1024
                for hh in range(2):
                    po = 64 * hh
                    nc.tensor.matmul(
                        P[:, base + hh * 512: base + hh * 512 + 512],
                        lhsT=kTm[mt][po:po + 64, kb * 128:(kb + 1) * 128],
                        rhs=qTm[mt][po:po + 64, qc * 512:(qc + 1) * 512],
                        start=True, stop=True,
                    )
                nc.scalar.activation(ptt[p % NPT][:], P[:, base:base + 1024],
                                     AF.Exp, scale=SCALE)
            # weave
            for fn in W.pop(p, ()):
                fn()

        for p in range(nsteps + LAG):
            emit_step(p)

        # ---------------- tail: rows 12..15 ----------
        # Pre-run the atm[0] halves (ready since chunk 6's norm) while the
        # last norm chain drains; only the atm[1] halves + copies wait on
        # norm2(7). Accumulation groups stay open across banks 0/2/4/6.
        tcols = (0, 1024, 2048, 3072)
        for j, st in enumerate((12, 13, 14, 15)):
            col = tcols[j]
            for ec in range(2):
                nc.tensor.matmul(
                    P[:, col + ec * 512:col + ec * 512 + 512],
                    lhsT=atm[0][:, st * 128:(st + 1) * 128],
                    rhs=wo_sb[:, 0, ec * 512:(ec + 1) * 512],
                    start=True, stop=False,
                )
        norm2(7)
        for j, st in enumerate((12, 13, 14, 15)):
            col = tcols[j]
            for ec in range(2):
                nc.tensor.matmul(
                    P[:, col + ec * 512:col + ec * 512 + 512],
                    lhsT=atm[1][:, st * 128:(st + 1) * 128],
                    rhs=wo_sb[:, 1, ec * 512:(ec + 1) * 512],
                    start=False, stop=True,
                )
            r = yo_i[0] % 3
            yo_i[0] += 1
            if st % 2 == 1:
                nc.scalar.activation(yo[r][:], P[:, col:col + 1024], AF.Copy)
            else:
                nc.vector.tensor_copy(yo[r][:], P[:, col:col + 1024])
            nc.sync.dma_start(y[st * 128:(st + 1) * 128, 0:512],
                              yo[r][:, 0:512])
            nc.scalar.dma_start(y[st * 128:(st + 1) * 128, 512:1024],
                                yo[r][:, 512:1024])
        norm2_at.clear()

    nc.finalize()
    return nc


def get_nc():
    if "nc" not in _CACHE:
        _CACHE["nc"] = _build_nc()
    return _CACHE["nc"]


def make_in_maps(query, key, value, W_q, W_k, W_v, W_o):
    bf = ml_dtypes.bfloat16

    def t(a):  # contiguous transpose + bf16 cast
        return np.ascontiguousarray(np.asarray(a, np.float32).T).astype(bf)

    xq = {b: t(query[b]) for b in range(B)}
    xk = {b: t(key[b]) for b in range(B)}
    xv = {b: t(value[b]) for b in range(B)}
    W_q, W_k, W_v, W_o = (np.asarray(w, np.float32) for w in (W_q, W_k, W_v, W_o))
    wq = {g: t(W_q[g * DL:(g + 1) * DL, :]) for g in range(4)}
    wk = {g: t(W_k[g * DL:(g + 1) * DL, :]) for g in range(4)}
    wv = {g: t(W_v[g * DL:(g + 1) * DL, :]) for g in range(4)}
    wo = {g: t(W_o[:, g * DL:(g + 1) * DL]) for g in range(4)}

    in_maps = []
    for c in range(NCORES):
        b, g = divmod(c, 4)
        in_maps.append({
            "xqT": xq[b], "xkT": xk[b], "xvT": xv[b],
            "wqT": wq[g], "wkT": wk[g], "wvT": wv[g], "woT": wo[g],
        })
    return in_maps


def combine_outputs(results):
    """results: list of per-core dicts with 'y' -> full (B, S, D) output."""
    outs = [np.asarray(r["y"], np.float32) for r in results]
    return np.stack([
        outs[0] + outs[1] + outs[2] + outs[3],
        outs[4] + outs[5] + outs[6] + outs[7],
    ]).astype(np.float32)


def _exec_cached(nc, in_maps):
    """run_bass_via_pjrt with the jitted executable cached across calls."""
    import jax
    import jax.numpy as jnp  # noqa: F401
    from jax.sharding import Mesh, PartitionSpec
    from jax.experimental.shard_map import shard_map
    import concourse.mybir as mybir
    from concourse import bass2jax

    if "exec" not in _CACHE:
        bass2jax.install_neuronx_cc_hook()
        partition_name = (nc.partition_id_tensor.name
                          if nc.partition_id_tensor else None)
        in_names, out_names, out_avals = [], [], []
        for alloc in nc.m.functions[0].allocations:
            if not isinstance(alloc, mybir.MemoryLocationSet):
                continue
            name = alloc.memorylocations[0].name
            if alloc.kind == "ExternalInput":
                if name != partition_name:
                    in_names.append(name)
            elif alloc.kind == "ExternalOutput":
                out_avals.append(jax.core.ShapedArray(
                    tuple(alloc.tensor_shape), mybir.dt.np(alloc.dtype)))
                out_names.append(name)
        n_params = len(in_names)
        all_names = in_names + out_names
        if partition_name is not None:
            all_names.append(partition_name)
        donate = tuple(range(n_params, n_params + len(out_names)))

        def _body(*args):
            operands = list(args)
            if partition_name is not None:
                operands.append(bass2jax.partition_id_tensor())
            outs = bass2jax._bass_exec_p.bind(
                *operands,
                out_avals=tuple(out_avals),
                in_names=tuple(all_names),
                out_names=tuple(out_names),
                lowering_input_output_aliases=(),
                sim_require_finite=True,
                sim_require_nnan=True,
                nc=nc,
            )
            return tuple(outs)

        mesh = Mesh(np.asarray(jax.devices()[:NCORES]), ("core",))
        specs = (PartitionSpec("core"),) * (n_params + len(out_names))
        out_specs = (PartitionSpec("core"),) * len(out_names)
        _CACHE["exec"] = (
            jax.jit(shard_map(_body, mesh=mesh, in_specs=specs,
                              out_specs=out_specs, check_rep=False),
                    donate_argnums=donate, keep_unused=True),
            in_names, out_names, out_avals,
        )

    sharded, in_names, out_names, out_avals = _CACHE["exec"]
    concat_in = [
        np.concatenate([np.asarray(in_maps[c][name]) for c in range(NCORES)],
                       axis=0)
        for name in in_names
    ]
    concat_zeros = [
        np.zeros((NCORES * a.shape[0], *a.shape[1:]), a.dtype)
        for a in out_avals
    ]
    out_arrs = sharded(*concat_in, *concat_zeros)
    return [
        {name: np.asarray(out_arrs[i]).reshape(
            NCORES, *out_avals[i].shape)[c]
         for i, name in enumerate(out_names)}
        for c in range(NCORES)
    ]


def kernel(query, key, value, W_q, W_k, W_v, W_o):
    nc = get_nc()
    in_maps = make_in_maps(query, key, value, W_q, W_k, W_v, W_o)
    try:
        results = _exec_cached(nc, in_maps)
    except Exception:
        from concourse.bass_utils import run_bass_kernel_spmd
        _CACHE.pop("exec", None)
        results = run_bass_kernel_spmd(nc, in_maps, list(range(NCORES))).results
    return combine_outputs(results)
